# revision 17
# baseline (speedup 1.0000x reference)
"""DispersionLoss kernel for Trainium2 (8 NeuronCores, Bass/Tile).

Reference computation (N=16384, F=64, K=32, C=128):
    bin_mass[f,k]  = sum_n m[n,f,k] + EPS
    SWY[f,k,c]     = sum_n m[n,f,k] * y[n,c]
    cent[f,k,c]    = SWY / bin_mass
    loss_dispersion= sum_fk ( A/bin_mass - c_sq )     (algebraic expansion;
        A[f,k] = sum_n m[n,f,k]*|y_n|^2, the EPS cross-term is O(1e-11))
    loss_entropy   = sum_fk p*log(p+EPS), p = bin_mass/N
    loss_repulsion = sum_f sum_k exp(-|cent[f,k]-cent[f,k+1]|^2)
    loss_inter     = sum_f (sum_{kj} exp(-pairwise) - K) / 2 / F   (symmetry)

Sharding: over F (8 features per core) -> every loss term decomposes per-f,
so no cross-core collectives are needed; host sums 8 partial scalars.

Phase 1 (transposed vs v1): stationary = membership block (fp8), moving =
YE = [Y | 1 | ysq] (fp8, 130 cols, ysq host-computed from fp32 y).  Output
accumulates bin-major (128 bins x 130) per half directly in PSUM, so mass/A
cost 2 extra moving columns instead of a second matmul, and phase 2 needs no
transpose.  DoubleRow perf mode contracts 256 samples per matmul: 64 blocks
x 2 halves = 128 matmuls total, each 130*0.5 PE cycles.  The kernel is then
HBM-bound: 4.2MB (G) + 2.1MB (YE) per core.

Phase 2: per-bin stats vectorized across both 128-bin halves, mean-centered
all-pairs distance stage, raw sums DMA'd out; the host sums the 8 cores'
partials and applies the final linear combines in fp64.
"""

import numpy as np

N = 16384
F = 64
K = 32
C = 128
NCORES = 8
F_PER_CORE = F // NCORES          # 8
FK = F_PER_CORE * K               # 256 bins per core
NB2 = N // 256                    # 64 double-row blocks (256 samples each)
YW = C + 2                        # 130: [Y | 1 | ysq]
GB = 2 * FK                       # 512 G cols per block (h-major, pair, m)
YB = 2 * YW                       # 260 YE cols per block

LAMBDA_ENTROPY = 0.1
LAMBDA_REPULSION = 0.5
LAMBDA_INTER = 0.3
EPS = 1e-8

GST = 8                           # blocks per G DMA super-tile (8 tiles)
YST = 16                          # blocks per YE DMA chunk (4 chunks)

_NC_CACHE = {}


def _np_f8():
    import ml_dtypes
    return ml_dtypes.float8_e4m3


def _pack_g(gc: np.ndarray) -> np.ndarray:
    """(N, FK) fp8 -> (128, NB2*GB): col b*512 + h*256 + i*128 + m holds
    gc[b*256 + i*128 + k, h*128 + m] at partition k (DoubleRow pairs)."""
    arr = gc.reshape(NB2, 2, 128, 2, 128)        # [b, i, k, h, m]
    arr = arr.transpose(2, 0, 3, 1, 4)           # [k, b, h, i, m]
    return np.ascontiguousarray(arr.reshape(128, NB2 * GB))


def _pack_ye(ye: np.ndarray) -> np.ndarray:
    """(N, YW) fp8 -> (128, NB2*YB): col b*260 + i*130 + c holds
    ye[b*256 + i*128 + k, c] at partition k."""
    arr = ye.reshape(NB2, 2, 128, YW)            # [b, i, k, c]
    arr = arr.transpose(2, 0, 1, 3)              # [k, b, i, c]
    return np.ascontiguousarray(arr.reshape(128, NB2 * YB))


def _finalize(parts: np.ndarray):
    """parts: (ncores, 8) raw per-core sums
    [wv0, wv1, ent0, ent1, en_tot, en_inv, e_allsum, 0]."""
    r = parts.astype(np.float64).sum(axis=0)
    disp = r[0] + r[1]
    ent = r[2] + r[3]
    rep = r[4] - r[5]
    inter = (r[6] - F * K) / (2.0 * F)
    tot = disp + LAMBDA_ENTROPY * ent + LAMBDA_REPULSION * rep + LAMBDA_INTER * inter
    return tuple(np.float32(v) for v in (tot, disp, ent, rep, inter))


def _build_nc():
    import concourse.bacc as bacc
    import concourse.tile as tile
    from concourse import mybir

    f32 = mybir.dt.float32
    bf16 = mybir.dt.bfloat16
    f8 = mybir.dt.float8e4

    nc = bacc.Bacc("TRN2", target_bir_lowering=False, debug=False,
                   enable_asserts=False, enable_partition_id=False)
    g_dram = nc.dram_tensor("g", (128, NB2 * GB), f8, kind="ExternalInput").ap()
    y_dram = nc.dram_tensor("y", (128, NB2 * YB), f8, kind="ExternalInput").ap()
    out_dram = nc.dram_tensor("out", (1, 8), f32, kind="ExternalOutput").ap()

    with tile.TileContext(nc) as tc:
        with (
            tc.tile_pool(name="singles", bufs=1) as singles,
            tc.tile_pool(name="scr", bufs=2) as scr,
            tc.tile_pool(name="ph2", bufs=1) as ph2,
            tc.tile_pool(name="psacc", bufs=1, space="PSUM") as psacc,
            tc.tile_pool(name="pstmp", bufs=2, space="PSUM") as pstmp,
            tc.tile_pool(name="pwq", bufs=2, space="PSUM") as pwq,
        ):
            # ---- streaming inputs: G fully resident (32KB/part fp8), YE
            # resident (16.25KB/part); DMA'd in super-tiles interleaved in
            # consumption order on the sync queue so the first matmuls start
            # as soon as YE chunk 0 + G tile 0 land.
            g_res = singles.tile([128, NB2 * GB], f8, name="gres")
            ye = singles.tile([128, NB2 * YB], f8, name="ye")
            for st in range(NB2 // GST):
                if st % 2 == 0:
                    yc = (st // 2) * YST * YB
                    nc.sync.dma_start(out=ye[:, yc:yc + YST * YB],
                                      in_=y_dram[:, yc:yc + YST * YB])
                cs = st * GST * GB
                nc.sync.dma_start(out=g_res[:, cs:cs + GST * GB],
                                  in_=g_dram[:, cs:cs + GST * GB])

            # ---- constants (gpsimd; off the critical DMA/PE path) ----
            ones128 = singles.tile([128, 1], f32)
            nc.gpsimd.memset(ones128, 1.0)
            eps128 = singles.tile([128, 1], f32)
            nc.gpsimd.memset(eps128, EPS)
            id128 = singles.tile([128, 128], f32)        # +identity
            nc.gpsimd.memset(id128, 0.0)
            nc.gpsimd.affine_select(
                out=id128, in_=id128,
                compare_op=mybir.AluOpType.not_equal,
                fill=1.0, base=0, pattern=[[-1, 128]], channel_multiplier=1,
            )
            mhalf128 = singles.tile([128, 1], f32)       # -0.5 centering bias
            nc.gpsimd.memset(mhalf128, -0.5)
            ones128_bf = singles.tile([128, 1], bf16)
            nc.gpsimd.memset(ones128_bf, 1.0)
            onesrow_bf = singles.tile([1, 128], bf16)
            nc.gpsimd.memset(onesrow_bf, 1.0)
            # diag-block masks (full width, for fused mul+reduce):
            # dmask[p, q*FK + j] = 1 iff j//K == q*4 + p//32
            dmask = singles.tile([128, 2 * FK], f32)
            nc.gpsimd.memset(dmask, 0.0)
            for q in range(2):
                for fl in range(4):
                    fg = (q * 4 + fl) * K
                    nc.gpsimd.memset(
                        dmask[32 * fl:32 * fl + 32,
                              q * FK + fg:q * FK + fg + K], 1.0)
            res = ph2.tile([1, 8], f32)
            nc.gpsimd.memset(res, 0.0)

            # ---- warm the ACT tables with the exact signatures phase 2
            # uses, so no 1.3us table load lands in the serial tail.
            warm = ph2.tile([1, 4], f32)
            nc.scalar.activation(out=warm[0:1, 0:1], in_=eps128[0:1, 0:1],
                                 func=mybir.ActivationFunctionType.Ln,
                                 bias=eps128[0:1, 0:1], scale=1.0)
            warm_acc = ph2.tile([1, 1], f32)
            nc.scalar.activation(out=warm[0:1, 1:2], in_=eps128[0:1, 0:1],
                                 func=mybir.ActivationFunctionType.Exp,
                                 scale=-1.0, accum_out=warm_acc)
            nc.scalar.activation(out=warm[0:1, 2:3], in_=eps128[0:1, 0:1],
                                 func=mybir.ActivationFunctionType.Exp,
                                 bias=eps128[0:1, 0:1], scale=2.0)

            # ---- phase 1: per block b: ps[h] += G[b,h]^T @ YE[b] ----
            ps = [psacc.tile([128, YW], f32, name=f"acc{h}") for h in range(2)]
            g3 = g_res.rearrange("p (b h two m) -> p b h two m",
                                 b=NB2, h=2, two=2)
            ye3 = ye.rearrange("p (b two c) -> p b two c", b=NB2, two=2)
            for b in range(NB2):
                for h in range(2):
                    nc.tensor.matmul(
                        ps[h], g3[:, b, h], ye3[:, b],
                        start=(b == 0), stop=(b == NB2 - 1),
                        perf_mode=mybir.MatmulPerfMode.DoubleRow,
                    )

            # ---- per-bin stats, halves vectorized as columns (128 x 2) ----
            mass2 = ph2.tile([128, 2], f32)
            for h in range(2):
                nc.vector.tensor_scalar_add(mass2[:, h:h + 1],
                                            in0=ps[h][:, C:C + 1], scalar1=EPS)
            inv2 = ph2.tile([128, 2], f32)
            nc.vector.reciprocal(inv2, mass2)
            cent = ph2.tile([128, FK], f32)
            for h in range(2):
                nc.vector.tensor_scalar_mul(
                    cent[:, h * 128:(h + 1) * 128],
                    in0=ps[h][:, 0:C], scalar1=inv2[:, h:h + 1],
                )
            csq_scr = scr.tile([128, FK], f32, tag="csqscr")
            nc.vector.tensor_mul(csq_scr, cent, cent)
            c_sq2 = ph2.tile([128, 2], f32)
            nc.vector.reduce_sum(
                c_sq2, csq_scr.rearrange("p (h c) -> p h c", c=128),
                axis=mybir.AxisListType.X,
            )
            # wv = A*inv - c_sq ; ent = p*ln(p+EPS)
            st = ph2.tile([128, 4], f32)
            t0 = ph2.tile([128, 2], f32)
            for h in range(2):
                nc.vector.tensor_mul(t0[:, h:h + 1], ps[h][:, C + 1:C + 2],
                                     inv2[:, h:h + 1])
            nc.vector.tensor_sub(st[:, 0:2], t0, c_sq2)
            pp2 = ph2.tile([128, 2], f32)
            nc.vector.tensor_scalar_mul(pp2, in0=mass2, scalar1=1.0 / N)
            lg2 = ph2.tile([128, 2], f32)
            nc.scalar.activation(
                out=lg2, in_=pp2,
                func=mybir.ActivationFunctionType.Ln,
                bias=eps128, scale=1.0,
            )
            nc.vector.tensor_mul(st[:, 2:4], pp2, lg2)
            ps_st = pstmp.tile([1, 4], f32, tag="pstmp")
            nc.tensor.matmul(ps_st, ones128, st, start=True, stop=True)

            # ---- pairs stage: transpose cent (c x fk), center at the
            # constant 0.5 (centroids of uniform data sit at ~0.5; distances
            # are shift-invariant) so everything downstream is small enough
            # for bf16 matmuls at full PE rate.
            ps_cc = pwq.tile([128, FK], f32, tag="pwq", name="pscc")
            for h in range(2):
                nc.tensor.matmul(ps_cc[:, h * 128:(h + 1) * 128],
                                 cent[:, h * 128:(h + 1) * 128], id128,
                                 start=True, stop=True)
            cc_bf = ph2.tile([128, FK], bf16)
            with nc.allow_low_precision(reason="centered centroids ~1e-3"):
                nc.vector.tensor_scalar_add(cc_bf, in0=ps_cc, scalar1=-0.5)
                cc2s = scr.tile([128, FK], bf16, tag="cc2s")
                nc.vector.tensor_mul(cc2s, cc_bf, cc_bf)
            ps_ccr = pstmp.tile([1, FK], f32, tag="pstmp")
            nc.tensor.matmul(ps_ccr, ones128_bf, cc2s, start=True, stop=True)
            ccr_sb = ph2.tile([1, FK], bf16)
            with nc.allow_low_precision(reason="centered csq ~1e-3"):
                nc.vector.tensor_copy(ccr_sb, ps_ccr)
                botr = ph2.tile([1, FK], bf16)
                nc.vector.tensor_scalar_mul(botr, in0=ccr_sb, scalar1=-0.5)
            # cq as negated columns for the exp bias
            ncq2 = ph2.tile([128, 2], f32)
            for q in range(2):
                ps_cq = pstmp.tile([128, 1], f32, tag="pstmp", name=f"pq{q}")
                nc.tensor.matmul(ps_cq, ccr_sb[0:1, q * 128:(q + 1) * 128],
                                 ones128_bf[0:1, 0:1], start=True, stop=True)
                nc.vector.tensor_scalar_mul(ncq2[:, q:q + 1], in0=ps_cq,
                                            scalar1=-1.0)

            # ---- repulsion: adjacent-bin distances from cc_bf ----
            with nc.allow_low_precision(reason="adjacent deltas ~1e-3"):
                dd = ph2.tile([128, FK - 1], bf16)
                nc.vector.tensor_sub(dd, cc_bf[:, 0:FK - 1], cc_bf[:, 1:FK])
                nc.vector.tensor_mul(dd, dd, dd)
            ps_nd = pstmp.tile([1, FK - 1], f32, tag="pstmp")
            nc.tensor.matmul(ps_nd, ones128_bf, dd, start=True, stop=True)
            en = ph2.tile([1, FK - 1], f32)
            en_tot = ph2.tile([1, 1], f32)
            nc.scalar.activation(
                out=en, in_=ps_nd, func=mybir.ActivationFunctionType.Exp,
                scale=-1.0, accum_out=en_tot,
            )
            inv_view = en[0:1, 0:(F_PER_CORE - 1) * K].rearrange(
                "p (a b) -> p a b", b=K
            )[:, :, K - 1:K]
            inv_sum = ph2.tile([1, 1], f32)
            nc.vector.reduce_sum(inv_sum, inv_view, axis=mybir.AxisListType.XY)

            # ---- inter: E = exp(2*dots - cq_j (rank-1 mm) - cq_k (ACT
            # bias)); reduce only the diagonal (same-f) blocks via masks.
            erows = ph2.tile([128, 2], f32)
            for q in range(2):
                psq = pwq.tile([128, FK], f32, tag="pwq", name=f"psq{q}")
                nc.tensor.matmul(psq, cc_bf[:, q * 128:(q + 1) * 128], cc_bf,
                                 start=True, stop=False)
                nc.tensor.matmul(psq, onesrow_bf, botr,
                                 start=False, stop=True)
                e_full = scr.tile([128, FK], f32, tag="efull", name=f"ef{q}")
                nc.scalar.activation(
                    out=e_full, in_=psq,
                    func=mybir.ActivationFunctionType.Exp, scale=2.0,
                    bias=ncq2[:, q:q + 1],
                )
                emask = scr.tile([128, FK], f32, tag="emask", name=f"emk{q}")
                nc.vector.tensor_mul(emask, e_full,
                                     dmask[:, q * FK:(q + 1) * FK])
                nc.vector.reduce_sum(erows[:, q:q + 1], emask,
                                     axis=mybir.AxisListType.X)
            ecol = ph2.tile([128, 1], f32)
            nc.vector.reduce_sum(ecol, erows, axis=mybir.AxisListType.X)
            ps_i = pstmp.tile([1, 1], f32, tag="pstmp")
            nc.tensor.matmul(ps_i, ones128, ecol, start=True, stop=True)

            # ---- raw outputs; host finishes the linear combines ----
            # res = [wv0, wv1, ent0, ent1, en_tot, en_inv, e_allsum, 0]
            nc.vector.tensor_copy(res[0:1, 0:4], ps_st)
            nc.vector.tensor_copy(res[0:1, 4:5], en_tot)
            nc.vector.tensor_copy(res[0:1, 5:6], inv_sum)
            nc.vector.tensor_copy(res[0:1, 6:7], ps_i)
            nc.sync.dma_start(out=out_dram, in_=res)

    nc.compile()
    return nc


def get_nc():
    if "f8" not in _NC_CACHE:
        _NC_CACHE["f8"] = _build_nc()
    return _NC_CACHE["f8"]


def kernel(membership: np.ndarray, teacher_preds: np.ndarray, _trace: bool = False):
    from concourse.bass_utils import run_bass_kernel_spmd

    f8 = _np_f8()
    m = np.asarray(membership, dtype=np.float32).reshape(N, F * K)
    y32 = np.asarray(teacher_preds, dtype=np.float32)
    ysq = np.sum(y32 * y32, axis=1, keepdims=True)
    ye = np.concatenate(
        [y32, np.ones((N, 1), dtype=np.float32), ysq], axis=1).astype(f8)
    ye_pack = _pack_ye(ye)

    nc = get_nc()
    in_maps = []
    for i in range(NCORES):
        in_maps.append({
            "g": _pack_g(m[:, i * FK:(i + 1) * FK].astype(f8)),
            "y": ye_pack,
        })
    res = run_bass_kernel_spmd(
        nc, in_maps, core_ids=list(range(NCORES)), trace=_trace,
    )
    parts = np.stack(
        [np.asarray(res.results[i]["out"][0], dtype=np.float64) for i in range(NCORES)]
    )
    out = _finalize(parts)
    if _trace:
        return out, res
    return out


if __name__ == "__main__":
    rng = np.random.default_rng(0)
    mem = rng.random((N, F, K), dtype=np.float32)
    tp = rng.random((N, C), dtype=np.float32)
    print(kernel(mem, tp))
